# revision 1
# baseline (speedup 1.0000x reference)
"""Masked per-channel MAE generator loss on 8 trn2 NeuronCores.

Full inputs:
  out_labels    (16,1,30,30) f32
  out_images    (16,3,512,512) f32
  target_images (16,3,512,512) f32
  epoch         scalar int

Sharding: batch dim 16 -> 2 images per core. Each core streams its
6 channels (2 images x 3ch) of 512*512 = 128x2048 f32 pixels for both
out_images and target_images, producing per-partition partials:
  osum[128, 6*NCHUNK]  per-(channel,chunk) sum of |out - tgt|
  omax[128, 6*NCHUNK]  per-(channel,chunk) max of |tgt|   (validity)
  olab[1,1]            sum of this core's 1800 labels
Host finishes the tiny reduction exactly like the reference.
"""

import sys

if "/opt/trn_rl_repo" not in sys.path:
    sys.path.insert(0, "/opt/trn_rl_repo")

import numpy as np

N_CORES = 8
B = 16
PAIRS_PER_CORE = B // N_CORES          # 2
CH_PER_CORE = PAIRS_PER_CORE * 3       # 6
PIX = 512 * 512                        # 262144 per channel
P = 128
COLS = PIX // P                        # 2048
W = 512                                # columns per chunk
NCHUNK = COLS // W                     # 4
LBL_PER_CORE = PAIRS_PER_CORE * 900    # 1800

_cache = {}


def _build(reps=1):
    from concourse import bass, mybir

    f32 = mybir.dt.float32
    X = mybir.AxisListType.X
    nc = bass.Bass()
    nchunks = CH_PER_CORE * NCHUNK
    pair = nc.declare_dram_parameter("pair", [nchunks, P, 2 * W], f32, isOutput=False)
    lbl = nc.declare_dram_parameter("lbl", [1, LBL_PER_CORE], f32, isOutput=False)
    osum = nc.declare_dram_parameter("osum", [P, nchunks], f32, isOutput=True)
    omax = nc.declare_dram_parameter("omax", [P, nchunks], f32, isOutput=True)
    olab = nc.declare_dram_parameter("olab", [1, 1], f32, isOutput=True)

    qs = [nc.alloc_semaphore(f"qs{i}") for i in range(nchunks)]
    lbl_sem = nc.alloc_semaphore("lbl_sem")
    vdone = nc.alloc_semaphore("vdone")
    outs_sem = nc.alloc_semaphore("outs_sem")

    tp = [nc.alloc_sbuf_tensor(f"tp{i}", [P, 2 * W], f32) for i in range(nchunks)]
    ND = 4
    td = [nc.alloc_sbuf_tensor(f"td{j}", [P, W], f32) for j in range(ND)]
    sums = nc.alloc_sbuf_tensor("sums", [P, nchunks], f32)
    maxs = nc.alloc_sbuf_tensor("maxs", [P, nchunks], f32)
    tlb = nc.alloc_sbuf_tensor("tlb", [1, LBL_PER_CORE], f32)
    tls = nc.alloc_sbuf_tensor("tls", [1, 1], f32)

    with nc.Block() as block:

        @block.sync
        def _(sync: bass.BassEngine):
            for r in range(reps):
                if r > 0:
                    sync.wait_ge(vdone, r)
                for i in range(0, nchunks, 2):
                    sync.dma_start(out=tp[i][:], in_=pair[i]).then_inc(qs[i], 16)
                sync.dma_start(out=tlb[:], in_=lbl[:]).then_inc(lbl_sem, 16)
            sync.wait_ge(vdone, reps)
            sync.dma_start(out=osum[:], in_=sums[:]).then_inc(outs_sem, 16)
            sync.dma_start(out=omax[:], in_=maxs[:]).then_inc(outs_sem, 16)
            sync.dma_start(out=olab[:], in_=tls[:]).then_inc(outs_sem, 16)
            sync.wait_ge(outs_sem, 48)

        @block.scalar
        def _(scalar: bass.BassEngine):
            for r in range(reps):
                if r > 0:
                    scalar.wait_ge(vdone, r)
                for i in range(1, nchunks, 2):
                    scalar.dma_start(out=tp[i][:], in_=pair[i]).then_inc(qs[i], 16)

        @block.vector
        def _(vector: bass.BassEngine):
            for r in range(reps):
                for i in range(nchunks):
                    vector.wait_ge(qs[i], 16 * (r + 1))
                    vector.tensor_sub(td[i % ND][:], tp[i][:, 0:W], tp[i][:, W:2 * W])
                    vector.reduce_max(
                        out=maxs[:, i:i + 1], in_=tp[i][:, W:2 * W],
                        axis=X, apply_absolute_value=True,
                    )
                    vector.reduce_sum(
                        out=sums[:, i:i + 1], in_=td[i % ND][:],
                        axis=X, apply_absolute_value=True,
                    )
                vector.wait_ge(lbl_sem, 16 * (r + 1))
                vector.reduce_sum(out=tls[:], in_=tlb[:], axis=X).then_inc(vdone, 1)

    return nc


def _get_nc():
    if "nc" not in _cache:
        _cache["nc"] = _build()
    return _cache["nc"]


def run_on_cores(out_labels, out_images, target_images, trace=False):
    """Shard, execute on 8 cores, return (results_list, exec_time_ns)."""
    from concourse.bass_utils import run_bass_kernel_spmd

    nc = _get_nc()
    out_images = np.ascontiguousarray(out_images, dtype=np.float32)
    target_images = np.ascontiguousarray(target_images, dtype=np.float32)
    out_labels = np.ascontiguousarray(out_labels, dtype=np.float32)
    nchunks = CH_PER_CORE * NCHUNK
    in_maps = []
    for i in range(N_CORES):
        sl = slice(i * PAIRS_PER_CORE, (i + 1) * PAIRS_PER_CORE)
        packed = np.concatenate(
            [
                out_images[sl].reshape(nchunks, P, W),
                target_images[sl].reshape(nchunks, P, W),
            ],
            axis=2,
        )
        in_maps.append({
            "pair": np.ascontiguousarray(packed),
            "lbl": np.ascontiguousarray(out_labels[sl].reshape(1, LBL_PER_CORE)),
        })
    res = run_bass_kernel_spmd(nc, in_maps, core_ids=list(range(N_CORES)), trace=trace)
    return res.results, getattr(res, "exec_time_ns", None)


def combine(results, epoch):
    sums8 = np.stack([np.asarray(r["osum"]) for r in results])  # [8,128,6*NCHUNK]
    maxs8 = np.stack([np.asarray(r["omax"]) for r in results])
    lab = np.float32(sum(float(np.asarray(r["olab"]).ravel()[0]) for r in results))

    abs_sum = sums8.reshape(N_CORES, P, CH_PER_CORE, NCHUNK).sum(axis=(1, 3))
    mx = maxs8.reshape(N_CORES, P, CH_PER_CORE, NCHUNK).max(axis=(1, 3))
    per_ch_mae = (abs_sum.astype(np.float32) / np.float32(PIX)).reshape(B, 3)
    valid_f = (mx > 0).astype(np.float32).reshape(B, 3)
    cnt = valid_f.sum(axis=1)
    tot = (per_ch_mae * valid_f).sum(axis=1)
    pair = np.where(cnt > 0, tot / np.maximum(cnt, np.float32(1.0)), np.float32(0.0))
    image_loss = pair.mean(dtype=np.float32)
    adv = -(lab / np.float32(B * 900))
    ep = int(np.asarray(epoch).ravel()[0]) if not isinstance(epoch, int) else epoch
    return np.float32(image_loss + np.float32(0.01) * adv / np.float32(ep + 1))


def kernel(out_labels, out_images, target_images, epoch):
    results, _ = run_on_cores(out_labels, out_images, target_images, trace=False)
    return combine(results, epoch)



# revision 4
# speedup vs baseline: 1.7428x; 1.7428x over previous
"""Masked per-channel MAE generator loss on 8 trn2 NeuronCores.

Full inputs:
  out_labels    (16,1,30,30) f32
  out_images    (16,3,512,512) f32
  target_images (16,3,512,512) f32
  epoch         scalar int

Sharding: batch dim 16 -> 2 images per core (data parallel). Host
converts images to bf16 and packs, per core, 6 channel tiles of
[128, 4096] = [out_ch | tgt_ch] (each [128, 2048]). Device pipeline
per channel i:
  vector: d = out - tgt                     (tensor_tensor, 2x bf16)
          acc[:,6+i] = count(tgt != 0)      (tensor_scalar reduce, 4x)
  scalar: acc[:,i]   = sum |d|              (ACT Abs with accumulator)
plus a label-sum column on vector. One [128,16] f32 result tile per
core; host finishes the tiny reduction exactly like the reference.
"""

import sys

if "/opt/trn_rl_repo" not in sys.path:
    sys.path.insert(0, "/opt/trn_rl_repo")

import numpy as np

N_CORES = 8
B = 16
PAIRS_PER_CORE = B // N_CORES          # 2
CH_PER_CORE = PAIRS_PER_CORE * 3       # 6
PIX = 512 * 512                        # 262144 per channel
P = 128
COLS = PIX // P                        # 2048
LBL_PER_CORE = PAIRS_PER_CORE * 900    # 1800
LBL_COLS = 15                          # 128*15 = 1920 >= 1800 (zero padded)
NACC = 16                              # acc columns: 6 sums, 6 validity, 1 label

_cache = {}


def _build():
    from concourse import bass, mybir

    f32 = mybir.dt.float32
    bf16 = mybir.dt.bfloat16
    NE = mybir.AluOpType.not_equal
    ADD = mybir.AluOpType.add
    ABS = mybir.ActivationFunctionType.Abs
    nc = bass.Bass()

    pair = nc.declare_dram_parameter(
        "pair", [CH_PER_CORE, P, 2 * COLS], bf16, isOutput=False
    )
    lbl = nc.declare_dram_parameter("lbl", [P, LBL_COLS], f32, isOutput=False)
    oacc = nc.declare_dram_parameter("oacc", [P, NACC], f32, isOutput=True)

    qs = nc.alloc_semaphore("qs")
    vsub = nc.alloc_semaphore("vsub")
    adone = nc.alloc_semaphore("adone")
    vdone = nc.alloc_semaphore("vdone")
    outs_sem = nc.alloc_semaphore("outs_sem")

    buf = [
        nc.alloc_sbuf_tensor(f"buf{i}", [P, 2 * COLS], bf16)
        for i in range(CH_PER_CORE)
    ]
    d = [nc.alloc_sbuf_tensor(f"d{j}", [P, COLS], bf16) for j in range(2)]
    vscr = nc.alloc_sbuf_tensor("vscr", [P, COLS], bf16)
    ascr = nc.alloc_sbuf_tensor("ascr", [P, COLS], bf16)
    acc = nc.alloc_sbuf_tensor("acc", [P, NACC], f32)
    lblbuf = nc.alloc_sbuf_tensor("lblbuf", [P, LBL_COLS], f32)
    lscr = nc.alloc_sbuf_tensor("lscr", [P, LBL_COLS], f32)

    with nc.Block() as block:

        @block.sync
        def _(sync: bass.BassEngine):
            for i in range(CH_PER_CORE):
                sync.dma_start(out=buf[i][:], in_=pair[i]).then_inc(qs, 16)
            sync.dma_start(out=lblbuf[:], in_=lbl[:]).then_inc(qs, 16)
            sync.wait_ge(vdone, 1)
            sync.wait_ge(adone, CH_PER_CORE)
            sync.dma_start(out=oacc[:], in_=acc[:]).then_inc(outs_sem, 16)
            sync.wait_ge(outs_sem, 16)

        @block.vector
        def _(vector: bass.BassEngine):
            for i in range(CH_PER_CORE):
                vector.wait_ge(qs, 16 * (i + 1))
                if i >= 2:
                    # d[i%2] still holds channel i-2 until ACT consumed it
                    vector.wait_ge(adone, i - 1)
                vector.tensor_sub(
                    d[i % 2][:], buf[i][:, 0:COLS], buf[i][:, COLS:2 * COLS]
                ).then_inc(vsub, 1)
                vector.tensor_scalar(
                    out=vscr[:], in0=buf[i][:, COLS:2 * COLS], scalar1=0.0,
                    scalar2=None, op0=NE, op1=ADD,
                    accum_out=acc[:, 6 + i:7 + i],
                )
            vector.wait_ge(qs, 16 * (CH_PER_CORE + 1))
            vector.tensor_scalar(
                out=lscr[:], in0=lblbuf[:], scalar1=0.0, scalar2=None,
                op0=ADD, op1=ADD, accum_out=acc[:, 12:13],
            ).then_inc(vdone, 1)

        @block.scalar
        def _(scalar: bass.BassEngine):
            for i in range(CH_PER_CORE):
                scalar.wait_ge(vsub, i + 1)
                scalar.activation(
                    out=ascr[:], in_=d[i % 2][:], func=ABS,
                    accum_out=acc[:, i:i + 1],
                ).then_inc(adone, 1)

    return nc


def _get_nc():
    if "nc" not in _cache:
        _cache["nc"] = _build()
    return _cache["nc"]


def _pack_inputs(out_labels, out_images, target_images):
    import ml_dtypes

    bf16 = ml_dtypes.bfloat16
    out_bf = np.asarray(out_images, dtype=np.float32).astype(bf16)
    tgt_bf = np.asarray(target_images, dtype=np.float32).astype(bf16)
    out_labels = np.ascontiguousarray(out_labels, dtype=np.float32)

    in_maps = []
    for c in range(N_CORES):
        sl = slice(c * PAIRS_PER_CORE, (c + 1) * PAIRS_PER_CORE)
        o = out_bf[sl].reshape(CH_PER_CORE, P, COLS)
        t = tgt_bf[sl].reshape(CH_PER_CORE, P, COLS)
        packed = np.concatenate([o, t], axis=2)  # [6, 128, 4096]
        lab = np.zeros((P, LBL_COLS), dtype=np.float32)
        lab.reshape(-1)[:LBL_PER_CORE] = out_labels[sl].reshape(-1)
        in_maps.append({
            "pair": np.ascontiguousarray(packed),
            "lbl": lab,
        })
    return in_maps


def run_on_cores(out_labels, out_images, target_images, trace=False):
    """Shard, execute on 8 cores, return (results_list, exec_time_ns)."""
    from concourse.bass_utils import run_bass_kernel_spmd

    nc = _get_nc()
    in_maps = _pack_inputs(out_labels, out_images, target_images)
    res = run_bass_kernel_spmd(nc, in_maps, core_ids=list(range(N_CORES)), trace=trace)
    return res.results, getattr(res, "exec_time_ns", None)


def combine(results, epoch):
    accs = np.stack([np.asarray(r["oacc"]) for r in results])  # [8,128,16]
    col = accs.sum(axis=1, dtype=np.float64)                   # [8,16]
    abs_sum = col[:, 0:6].reshape(B, 3)
    valid_f = (col[:, 6:12].reshape(B, 3) > 0).astype(np.float32)
    lab = col[:, 12].sum()

    per_ch_mae = (abs_sum / PIX).astype(np.float32)
    cnt = valid_f.sum(axis=1)
    tot = (per_ch_mae * valid_f).sum(axis=1)
    pair = np.where(cnt > 0, tot / np.maximum(cnt, np.float32(1.0)), np.float32(0.0))
    image_loss = pair.mean(dtype=np.float32)
    adv = -np.float32(lab / (B * 900))
    ep = int(np.asarray(epoch).ravel()[0]) if not isinstance(epoch, int) else epoch
    return np.float32(image_loss + np.float32(0.01) * adv / np.float32(ep + 1))


def kernel(out_labels, out_images, target_images, epoch):
    results, _ = run_on_cores(out_labels, out_images, target_images, trace=False)
    return combine(results, epoch)
